# revision 14
# baseline (speedup 1.0000x reference)
"""Causal self-attention (B=4,T=2048,C=1024) on 8 TRN2 NeuronCores.

Sharding: core c = 2*b + h handles batch b and global q-blocks g = 2k+h
(k=0..7, 128 rows each; even-parity cores waste one fully-masked block
so the program is SPMD-uniform and load balanced). The kv projection is
split between the two cores of a pair: core h computes kT/v for s-half
h only, and the halves are exchanged via pairwise AllGather collectives
(DRAM bounce buffers) in 4 pipelined 1MB chunks. Phase order is
kT -> v -> q projection so the exchange latency hides behind ~55us of
projection compute before attention consumes the gathered kv. All
inputs are pre-rearranged on the host into partition-contiguous
layouts so every load DMA is descriptor-light.
"""

import math
import sys

for p in ("/opt/trn_rl_repo",):
    if p not in sys.path:
        sys.path.insert(0, p)

import numpy as np
import ml_dtypes

import concourse.bass as bass
import concourse.tile as tile
from concourse import mybir
from concourse.masks import make_identity
from concourse.bass_utils import run_bass_kernel_spmd

B, T, C = 4, 2048, 1024
P = 128
NQB = 8            # q-blocks per core
NCB = C // P       # 8 c-chunks (contraction for projections)
NDB = C // P       # 8 d-chunks (contraction for QK)
NSB = T // P       # 16 s-blocks
F32 = mybir.dt.float32
BF16 = mybir.dt.bfloat16
SCALE = 1.0 / math.sqrt(C)
NEG = -1e30
GROUPS = [[0, 1], [2, 3], [4, 5], [6, 7]]


def build_nc(jitter=0):
    nc = bass.Bass()
    # host-prearranged, partition-contiguous inputs (see _host_inputs)
    xq = nc.declare_dram_parameter("xq", [P, 2, NCB, 512], BF16, isOutput=False)
    xth = nc.declare_dram_parameter("xth", [P, 2, NCB, 512], BF16, isOutput=False)
    wq = nc.declare_dram_parameter("wq", [P, NDB, NCB, P], BF16, isOutput=False)
    wk = nc.declare_dram_parameter("wk", [P, NDB, NCB, P], BF16, isOutput=False)
    wv = nc.declare_dram_parameter("wv", [P, 2, NCB, 512], BF16, isOutput=False)
    mask = nc.declare_dram_parameter("mask", [P, 2 * P], BF16, isOutput=False)
    out = nc.declare_dram_parameter("out", [NQB * P, C], BF16, isOutput=True)

    from contextlib import ExitStack
    with tile.TileContext(nc) as tc, ExitStack() as ctx:
        singles = ctx.enter_context(tc.tile_pool(name="singles", bufs=1))
        qkv = ctx.enter_context(tc.tile_pool(name="qkv", bufs=1))
        stat = ctx.enter_context(tc.tile_pool(name="stat", bufs=6))
        psA = ctx.enter_context(tc.tile_pool(name="psA", bufs=5, space="PSUM"))
        psT = ctx.enter_context(tc.tile_pool(name="psT", bufs=2, space="PSUM"))
        dram = ctx.enter_context(tc.tile_pool(name="dram", bufs=1, space="DRAM"))
        proj_ctx = ExitStack()
        xpool = proj_ctx.enter_context(tc.tile_pool(name="xpool", bufs=1))
        wbuf = proj_ctx.enter_context(tc.tile_pool(name="wbuf", bufs=1))
        kstage = proj_ctx.enter_context(tc.tile_pool(name="kstage", bufs=1))
        vstage = proj_ctx.enter_context(tc.tile_pool(name="vstage", bufs=2))

        ident = singles.tile([P, P], BF16)
        make_identity(nc, ident)
        mask_sb = singles.tile([P, 2 * P], BF16)
        recip_all = singles.tile([P, NQB], F32)

        touch_scr = stat.tile([P, 2], F32, tag="touch")
        for _ in range(jitter):  # schedule perturbation for wait-audit retries
            nc.vector.tensor_copy(out=touch_scr, in_=touch_scr)

        wq_all = wbuf.tile([P, NDB, NCB, P], BF16, tag="wq_all")
        wk_all = wbuf.tile([P, NDB, NCB, P], BF16, tag="wk_all")
        wv_all = wbuf.tile([P, 2, NCB, 512], BF16, tag="wv_all")
        xq_sb = xpool.tile([P, 2, NCB, 512], BF16, tag="xq")
        xth_sb = xpool.tile([P, 2, NCB, 512], BF16, tag="xth")

        # persistent SBUF tensors
        qT_sb = qkv.tile([P, NDB, NQB * P], BF16)   # [d%128, d//128, t]  2MB
        kT_sb = qkv.tile([P, 4, NDB, 512], BF16)    # [d%128, s//512, d//128, s%512]
        v_sb = qkv.tile([P, NSB, C], BF16)          # [s%128, s//128, d]  4MB

        # load order: wk db0 + xth sq0 first (kT projection is first on PE)
        nc.gpsimd.dma_start(out=wk_all[:, 0], in_=wk[:, 0])
        nc.scalar.dma_start(out=xth_sb[:, 0], in_=xth[:, 0])
        # warmup collective: the first CC on a core costs ~37us (NRT/DGE
        # init); a tiny AllGather issued early absorbs that under the load +
        # kT-projection phase so the real exchange runs at ~5us+12us/MB.
        # Single-row source = one contiguous descriptor (a [P, n] slice
        # would be P tiny descriptors and stall the gpsimd queue ~7us).
        cin_w = dram.tile([1, P], BF16, tag="cin_w")
        cout_w = dram.tile([2, P], BF16, tag="cout_w")
        nc.gpsimd.dma_start(out=cin_w, in_=mask[0:1, 0:P])
        nc.gpsimd.collective_compute(
            "AllGather", mybir.AluOpType.bypass, replica_groups=GROUPS,
            ins=[cin_w.opt()], outs=[cout_w.opt()])
        for db in range(1, NDB):
            nc.gpsimd.dma_start(out=wk_all[:, db], in_=wk[:, db])
        nc.gpsimd.dma_start(out=xth_sb[:, 1], in_=xth[:, 1])
        nc.gpsimd.dma_start(out=wv_all, in_=wv[:, :])
        nc.gpsimd.dma_start(out=mask_sb, in_=mask[:, :])

        # ------- Phase KV: own s-half only; pairwise AllGather exchange -----
        # Own-half chunk c of kT/v is global chunk (rank*half + c) after the
        # gather. All CC triggers are emitted before any readback; the tile
        # scheduler hoists what it can and NRT executes the CCs back-to-back.
        readbacks = []

        def kt_chunk(sqi):
            kst = kstage.tile([P, NDB, 512], BF16, tag="kst")
            for db in range(NDB):
                ps = psA.tile([P, 512], F32, tag="ps")
                for cb in range(NCB):
                    nc.tensor.matmul(
                        ps, wk_all[:, db, cb, :], xth_sb[:, sqi, cb, :],
                        start=(cb == 0), stop=(cb == NCB - 1))
                nc.scalar.copy(out=kst[:, db, :], in_=ps)
            cin = dram.tile([P, NDB, 512], BF16, tag=f"cin_kt{sqi}")
            cout = dram.tile([2, P, NDB, 512], BF16, tag=f"cout_kt{sqi}")
            nc.gpsimd.dma_start(out=cin, in_=kst)
            nc.gpsimd.collective_compute(
                "AllGather", mybir.AluOpType.bypass, replica_groups=GROUPS,
                ins=[cin.opt()], outs=[cout.opt()])

            def readback():
                for r in range(2):
                    nc.gpsimd.dma_start(
                        out=kT_sb[:, r * 2 + sqi], in_=cout[r])
            readbacks.append(readback)

        def v_chunk(vj):
            vst = vstage.tile([P, 4, C], BF16, tag="vst")
            for s2 in range(4):
                sbl = vj * 4 + s2
                sq, so = divmod(sbl, 4)
                ps0 = psA.tile([P, 512], F32, tag="ps")
                ps1 = psA.tile([P, 512], F32, tag="ps")
                for cb in range(NCB):
                    for dh, ps in ((0, ps0), (1, ps1)):
                        nc.tensor.matmul(
                            ps, xth_sb[:, sq, cb, so * P:(so + 1) * P],
                            wv_all[:, dh, cb, :],
                            start=(cb == 0), stop=(cb == NCB - 1))
                nc.scalar.copy(out=vst[:, s2, 0:512], in_=ps0)
                nc.scalar.copy(out=vst[:, s2, 512:1024], in_=ps1)
            cin = dram.tile([P, 4, C], BF16, tag=f"cin_v{vj}")
            cout = dram.tile([2, P, 4, C], BF16, tag=f"cout_v{vj}")
            nc.gpsimd.dma_start(out=cin, in_=vst)
            nc.gpsimd.collective_compute(
                "AllGather", mybir.AluOpType.bypass, replica_groups=GROUPS,
                ins=[cin.opt()], outs=[cout.opt()])

            def readback():
                for r in range(2):
                    nc.gpsimd.dma_start(
                        out=v_sb[:, r * 8 + vj * 4:r * 8 + vj * 4 + 4, :],
                        in_=cout[r])
            readbacks.append(readback)

        kt_chunk(0)
        kt_chunk(1)
        nc.gpsimd.dma_start(out=wq_all, in_=wq[:, :])
        nc.gpsimd.dma_start(out=xq_sb, in_=xq[:, :])
        v_chunk(0)
        v_chunk(1)
        for rb in readbacks:
            rb()

        # ---------------- Phase Q: qT = (W_q^T @ xq) * scale ----------------
        for th in range(2):
            for db in range(NDB):
                ps = psA.tile([P, 512], F32, tag="ps")
                for cb in range(NCB):
                    nc.tensor.matmul(
                        ps, wq_all[:, db, cb, :], xq_sb[:, th, cb, :],
                        start=(cb == 0), stop=(cb == NCB - 1))
                nc.scalar.mul(
                    out=qT_sb[:, db, th * 512:(th + 1) * 512], in_=ps,
                    mul=SCALE)

        # release projection-phase SBUF (w/x/stages) before attention allocs
        proj_ctx.close()
        att = ctx.enter_context(tc.tile_pool(name="att", bufs=2))
        attT = ctx.enter_context(tc.tile_pool(name="attT", bufs=1))
        ybuf = ctx.enter_context(tc.tile_pool(name="ybuf", bufs=8))

        # ---------------- Phase ATT ----------------
        # Split into a QK/softmax/transpose pass (consumes kT, which lands
        # early) and an AV pass (consumes v, which lands last) so the tail of
        # the kv exchange hides behind ~30us of score computation.
        probsTs = []
        for k in range(NQB):
            L = 2 * k + 2
            cols = L * P
            nch = (cols + 511) // 512
            widths = [min(512, cols - c * 512) for c in range(nch)]
            probs = att.tile([P, NQB * 2 * P], BF16, tag="probs")
            mx = stat.tile([P, 8], F32, tag="mx")
            negmax = stat.tile([P, 1], F32, tag="negmax")
            sums = stat.tile([P, 8], F32, tag="sums")
            rsum = stat.tile([P, 1], F32, tag="rsum")
            lo = cols - 256
            ch0, off = divmod(lo, 512)
            pss = []
            for ch in range(nch):
                wd = widths[ch]
                ps = psA.tile([P, 512], F32, tag="ps")
                pss.append(ps)
                has_mask = ch == ch0
                for db in range(NDB):
                    nc.tensor.matmul(
                        ps[:, 0:wd], qT_sb[:, db, k * P:(k + 1) * P],
                        kT_sb[:, ch, db, 0:wd],
                        start=(db == 0),
                        stop=(not has_mask and db == NDB - 1))
                if has_mask:
                    # mask folded into the accumulation group: += ident.T @ mask
                    nc.tensor.matmul(
                        ps[:, off:off + 256], ident, mask_sb,
                        start=False, stop=True)
            for ch in range(nch):
                nc.vector.reduce_max(
                    out=mx[:, ch:ch + 1], in_=pss[ch][:, 0:widths[ch]],
                    axis=mybir.AxisListType.X)
            nc.vector.reduce_max(
                out=negmax, in_=mx[:, 0:nch], axis=mybir.AxisListType.X,
                negate=True)
            for ch in range(nch):
                nc.scalar.activation(
                    out=probs[:, ch * 512:ch * 512 + widths[ch]],
                    in_=pss[ch][:, 0:widths[ch]],
                    func=mybir.ActivationFunctionType.Exp,
                    bias=negmax, scale=1.0,
                    accum_out=sums[:, ch:ch + 1])
            probsT = attT.tile([P, L, P], BF16, tag=f"probsT{k}")
            probsTs.append(probsT)
            for j in range(L):
                pt = psT.tile([P, P], BF16)
                nc.tensor.transpose(pt, probs[:, j * P:(j + 1) * P], ident)
                nc.vector.tensor_copy(out=probsT[:, j, :], in_=pt)
            nc.vector.reduce_sum(
                out=rsum, in_=sums[:, 0:nch], axis=mybir.AxisListType.X)
            nc.vector.reciprocal(out=recip_all[:, k:k + 1], in_=rsum)

        for k in range(NQB):
            L = 2 * k + 2
            probsT = probsTs[k]
            y_sb = ybuf.tile([P, C], BF16, tag="y")
            for dh in range(2):
                py = psA.tile([P, 512], F32, tag="ps")
                for j in range(L):
                    nc.tensor.matmul(
                        py, probsT[:, j, :],
                        v_sb[:, j, dh * 512:(dh + 1) * 512],
                        start=(j == 0), stop=(j == L - 1))
                nc.scalar.activation(
                    out=y_sb[:, dh * 512:(dh + 1) * 512], in_=py,
                    func=mybir.ActivationFunctionType.Copy, bias=0.0,
                    scale=recip_all[:, k:k + 1])
            # y out on the Activation hwdge queue: follows the producing
            # activation in the same engine stream, avoiding gpsimd
            # head-of-line blocking behind collective readback waits
            nc.scalar.dma_start(out=out[k * P:(k + 1) * P, :], in_=y_sb)

    return nc


def _host_inputs(x, W):
    """Build per-core input maps (partition-contiguous prearranged)."""
    tril = np.where(
        np.arange(P)[None, :] <= np.arange(P)[:, None], 0.0, NEG
    ).astype(np.float32)
    mask_even = np.concatenate([tril, np.full((P, P), NEG, np.float32)], 1)
    mask_odd = np.concatenate([np.zeros((P, P), np.float32), tril], 1)
    Wb = W.astype(ml_dtypes.bfloat16)
    # wq/wk: [p, db, cb, 128];  wv: [p, dh, cb, 512]
    wq_r = np.ascontiguousarray(
        Wb[:, 0:C].reshape(NCB, P, NDB, P).transpose(1, 2, 0, 3))
    wk_r = np.ascontiguousarray(
        Wb[:, C:2 * C].reshape(NCB, P, NDB, P).transpose(1, 2, 0, 3))
    wv_r = np.ascontiguousarray(
        Wb[:, 2 * C:3 * C].reshape(NCB, P, 2, 512).transpose(1, 2, 0, 3))
    in_maps = []
    for c in range(8):
        b, h = divmod(c, 2)
        xb = x[b].astype(ml_dtypes.bfloat16)        # [T, C]
        qrows = np.concatenate(
            [np.arange((2 * k + h) * P, (2 * k + h + 1) * P) for k in range(NQB)])
        # xq: [p, th, cb, 512]
        xq_r = np.ascontiguousarray(
            xb[qrows].T.reshape(NCB, P, 2, 512).transpose(1, 2, 0, 3))
        # xth (own s-half, transposed): [p, sq, cb, 512]
        xth_r = np.ascontiguousarray(
            xb[h * 1024:(h + 1) * 1024].T.reshape(NCB, P, 2, 512).transpose(
                1, 2, 0, 3))
        in_maps.append({
            "xq": xq_r, "xth": xth_r,
            "wq": wq_r, "wk": wk_r, "wv": wv_r,
            "mask": (mask_even if h == 0 else mask_odd).astype(
                ml_dtypes.bfloat16),
        })
    return in_maps


def _gather(results):
    y = np.zeros((B, T, C), np.float32)
    for c in range(8):
        b, h = divmod(c, 2)
        yc = results[c]["out"]
        for k in range(NQB):
            g = 2 * k + h
            y[b, g * P:(g + 1) * P, :] = yc[k * P:(k + 1) * P, :]
    return y


_SKIP_TYPES = ("InstCall", "InstUnconditionalBranch")


def _wait_limit(inst):
    t = type(inst).__name__
    if t in _SKIP_TYPES:
        return None
    return 1


def _split_excess_waits(nc):
    """HW instruction structs carry few sync-wait slots (1 for compute,
    2 for pseudo-DMA). Move excess waits onto same-engine EventSemaphore
    instructions inserted just before the offender (engines execute their
    stream in order, so this preserves semantics)."""
    fix = 0
    for blk in nc.m.functions[0].blocks:
        out = []
        for inst in blk.instructions:
            lim = _wait_limit(inst)
            si = inst.sync_info
            waits = list(si.on_wait) if si and si.on_wait else []
            if lim is not None and len(waits) > lim:
                for w in waits[:-lim]:
                    fix += 1
                    e = mybir.InstEventSemaphore(
                        name=f"I-waitfix-{fix}", ins=[], outs=[],
                        sync_info=mybir.SyncInfo(on_wait=[w], on_update=[]))
                    e.engine = inst.engine
                    out.append(e)
                si.on_wait = waits[-lim:]
            out.append(inst)
        blk.instructions[:] = out
    return fix


def _audit_waits(nc):
    bad = []
    for blk in nc.m.functions[0].blocks:
        for inst in blk.instructions:
            lim = _wait_limit(inst)
            si = inst.sync_info
            nw = len(si.on_wait) if si and si.on_wait else 0
            if lim is not None and nw > lim:
                bad.append((type(inst).__name__, inst.name, nw))
    return bad


def build_nc_checked(max_tries=6):
    last = None
    for i in range(max_tries):
        nc = build_nc(jitter=i)
        _split_excess_waits(nc)
        bad = _audit_waits(nc)
        if not bad:
            return nc
        last = bad
    raise RuntimeError(f"could not find wait-feasible schedule: {last[:5]}")


_CACHED = {}


def kernel(x, W_kqv):
    x = np.asarray(x, np.float32)
    W = np.asarray(W_kqv, np.float32)
    if "nc" not in _CACHED:
        _CACHED["nc"] = build_nc_checked()
    nc = _CACHED["nc"]
    in_maps = _host_inputs(x, W)
    res = run_bass_kernel_spmd(nc, in_maps, core_ids=list(range(8)))
    return _gather(res.results)


if __name__ == "__main__":
    x = np.random.randn(B, T, C).astype(np.float32)
    W = (np.random.randn(C, 3 * C) * 0.02).astype(np.float32)
    y = kernel(x, W)
    print("kernel ran:", y.shape, y.dtype)


# revision 15
# speedup vs baseline: 1.1142x; 1.1142x over previous
"""Causal self-attention (B=4,T=2048,C=1024) on 8 TRN2 NeuronCores.

Sharding: core c = 2*b + h handles batch b and global q-blocks g = 2k+h
(k=0..7, 128 rows each; even-parity cores waste one fully-masked block
so the program is SPMD-uniform and load balanced). The kv projection is
split between the two cores of a pair: core h computes kT/v for s-half
h only, and the halves are exchanged via pairwise AllGather collectives
(DRAM bounce buffers) in 4 pipelined 1MB chunks. Phase order is
kT -> v -> q projection so the exchange latency hides behind ~55us of
projection compute before attention consumes the gathered kv. All
inputs are pre-rearranged on the host into partition-contiguous
layouts so every load DMA is descriptor-light.
"""

import math
import sys

for p in ("/opt/trn_rl_repo",):
    if p not in sys.path:
        sys.path.insert(0, p)

import numpy as np
import ml_dtypes

import concourse.bass as bass
import concourse.tile as tile
from concourse import mybir
from concourse.masks import make_identity
from concourse.bass_utils import run_bass_kernel_spmd

B, T, C = 4, 2048, 1024
P = 128
NQB = 8            # q-blocks per core
NCB = C // P       # 8 c-chunks (contraction for projections)
NDB = C // P       # 8 d-chunks (contraction for QK)
NSB = T // P       # 16 s-blocks
F32 = mybir.dt.float32
BF16 = mybir.dt.bfloat16
SCALE = 1.0 / math.sqrt(C)
NEG = -1e30
GROUPS = [[0, 1], [2, 3], [4, 5], [6, 7]]


def build_nc(jitter=0):
    nc = bass.Bass()
    # host-prearranged, partition-contiguous inputs (see _host_inputs)
    xq = nc.declare_dram_parameter("xq", [P, 2, NCB, 512], BF16, isOutput=False)
    xth = nc.declare_dram_parameter("xth", [P, 2, NCB, 512], BF16, isOutput=False)
    wq = nc.declare_dram_parameter("wq", [P, NDB, NCB, P], BF16, isOutput=False)
    wk = nc.declare_dram_parameter("wk", [P, NDB, NCB, P], BF16, isOutput=False)
    wv = nc.declare_dram_parameter("wv", [P, 2, NCB, 512], BF16, isOutput=False)
    mask = nc.declare_dram_parameter("mask", [P, 2 * P], BF16, isOutput=False)
    out = nc.declare_dram_parameter("out", [NQB * P, C], BF16, isOutput=True)

    from contextlib import ExitStack
    with tile.TileContext(nc) as tc, ExitStack() as ctx:
        singles = ctx.enter_context(tc.tile_pool(name="singles", bufs=1))
        qkv = ctx.enter_context(tc.tile_pool(name="qkv", bufs=1))
        stat = ctx.enter_context(tc.tile_pool(name="stat", bufs=6))
        psA = ctx.enter_context(tc.tile_pool(name="psA", bufs=5, space="PSUM"))
        psT = ctx.enter_context(tc.tile_pool(name="psT", bufs=2, space="PSUM"))
        dram = ctx.enter_context(tc.tile_pool(name="dram", bufs=1, space="DRAM"))
        proj_ctx = ExitStack()
        xpool = proj_ctx.enter_context(tc.tile_pool(name="xpool", bufs=1))
        wbuf = proj_ctx.enter_context(tc.tile_pool(name="wbuf", bufs=1))
        kstage = proj_ctx.enter_context(tc.tile_pool(name="kstage", bufs=1))
        vstage = proj_ctx.enter_context(tc.tile_pool(name="vstage", bufs=2))

        ident = singles.tile([P, P], BF16)
        make_identity(nc, ident)
        mask_sb = singles.tile([P, 2 * P], BF16)
        recip_all = singles.tile([P, NQB], F32)

        touch_scr = stat.tile([P, 2], F32, tag="touch")
        for _ in range(jitter):  # schedule perturbation for wait-audit retries
            nc.vector.tensor_copy(out=touch_scr, in_=touch_scr)

        wq_all = wbuf.tile([P, NDB, NCB, P], BF16, tag="wq_all")
        wk_all = wbuf.tile([P, NDB, NCB, P], BF16, tag="wk_all")
        wv_all = wbuf.tile([P, 2, NCB, 512], BF16, tag="wv_all")
        xq_sb = xpool.tile([P, 2, NCB, 512], BF16, tag="xq")
        xth_sb = xpool.tile([P, 2, NCB, 512], BF16, tag="xth")

        # persistent SBUF tensors
        qT_sb = qkv.tile([P, NDB, NQB * P], BF16)   # [d%128, d//128, t]  2MB
        kT_sb = qkv.tile([P, 4, NDB, 512], BF16)    # [d%128, s//512, d//128, s%512]
        v_sb = qkv.tile([P, NSB, C], BF16)          # [s%128, s//128, d]  4MB

        # load order: wk db0 + xth sq0 first (kT projection is first on PE)
        nc.gpsimd.dma_start(out=wk_all[:, 0], in_=wk[:, 0])
        nc.gpsimd.dma_start(out=xth_sb[:, 0], in_=xth[:, 0])
        # warmup collective: the first CC on a core costs ~37us (NRT/DGE
        # init); a tiny AllGather issued early absorbs that under the load +
        # kT-projection phase so the real exchange runs at ~5us+12us/MB.
        # Single-row source = one contiguous descriptor (a [P, n] slice
        # would be P tiny descriptors and stall the gpsimd queue ~7us).
        cin_w = dram.tile([1, P], BF16, tag="cin_w")
        cout_w = dram.tile([2, P], BF16, tag="cout_w")
        nc.gpsimd.dma_start(out=cin_w, in_=mask[0:1, 0:P])
        nc.gpsimd.collective_compute(
            "AllGather", mybir.AluOpType.bypass, replica_groups=GROUPS,
            ins=[cin_w.opt()], outs=[cout_w.opt()])
        for db in range(1, NDB):
            nc.gpsimd.dma_start(out=wk_all[:, db], in_=wk[:, db])
        nc.gpsimd.dma_start(out=xth_sb[:, 1], in_=xth[:, 1])
        nc.gpsimd.dma_start(out=wv_all, in_=wv[:, :])
        nc.gpsimd.dma_start(out=mask_sb, in_=mask[:, :])

        # ------- Phase KV: own s-half only; pairwise AllGather exchange -----
        # Own-half chunk c of kT/v is global chunk (rank*half + c) after the
        # gather. All CC triggers are emitted before any readback; the tile
        # scheduler hoists what it can and NRT executes the CCs back-to-back.
        readbacks = []

        def kt_chunk(sqi):
            kst = kstage.tile([P, NDB, 512], BF16, tag="kst")
            for db in range(NDB):
                ps = psA.tile([P, 512], F32, tag="ps")
                for cb in range(NCB):
                    nc.tensor.matmul(
                        ps, wk_all[:, db, cb, :], xth_sb[:, sqi, cb, :],
                        start=(cb == 0), stop=(cb == NCB - 1))
                nc.scalar.copy(out=kst[:, db, :], in_=ps)
            cin = dram.tile([P, NDB, 512], BF16, tag=f"cin_kt{sqi}")
            cout = dram.tile([2, P, NDB, 512], BF16, tag=f"cout_kt{sqi}")
            nc.gpsimd.dma_start(out=cin, in_=kst)
            nc.gpsimd.collective_compute(
                "AllGather", mybir.AluOpType.bypass, replica_groups=GROUPS,
                ins=[cin.opt()], outs=[cout.opt()])

            def readback():
                for r in range(2):
                    nc.gpsimd.dma_start(
                        out=kT_sb[:, r * 2 + sqi], in_=cout[r])
            readbacks.append(readback)

        def v_chunk(vj):
            vst = vstage.tile([P, 4, C], BF16, tag="vst")
            for s2 in range(4):
                sbl = vj * 4 + s2
                sq, so = divmod(sbl, 4)
                ps0 = psA.tile([P, 512], F32, tag="ps")
                ps1 = psA.tile([P, 512], F32, tag="ps")
                for cb in range(NCB):
                    for dh, ps in ((0, ps0), (1, ps1)):
                        nc.tensor.matmul(
                            ps, xth_sb[:, sq, cb, so * P:(so + 1) * P],
                            wv_all[:, dh, cb, :],
                            start=(cb == 0), stop=(cb == NCB - 1))
                nc.scalar.copy(out=vst[:, s2, 0:512], in_=ps0)
                nc.scalar.copy(out=vst[:, s2, 512:1024], in_=ps1)
            cin = dram.tile([P, 4, C], BF16, tag=f"cin_v{vj}")
            cout = dram.tile([2, P, 4, C], BF16, tag=f"cout_v{vj}")
            nc.gpsimd.dma_start(out=cin, in_=vst)
            nc.gpsimd.collective_compute(
                "AllGather", mybir.AluOpType.bypass, replica_groups=GROUPS,
                ins=[cin.opt()], outs=[cout.opt()])

            def readback():
                for r in range(2):
                    nc.gpsimd.dma_start(
                        out=v_sb[:, r * 8 + vj * 4:r * 8 + vj * 4 + 4, :],
                        in_=cout[r])
            readbacks.append(readback)

        kt_chunk(0)
        kt_chunk(1)
        nc.gpsimd.dma_start(out=wq_all, in_=wq[:, :])
        nc.gpsimd.dma_start(out=xq_sb, in_=xq[:, :])
        v_chunk(0)
        v_chunk(1)
        for rb in readbacks:
            rb()

        # ---------------- Phase Q: qT = (W_q^T @ xq) * scale ----------------
        for th in range(2):
            for db in range(NDB):
                ps = psA.tile([P, 512], F32, tag="ps")
                for cb in range(NCB):
                    nc.tensor.matmul(
                        ps, wq_all[:, db, cb, :], xq_sb[:, th, cb, :],
                        start=(cb == 0), stop=(cb == NCB - 1))
                nc.scalar.mul(
                    out=qT_sb[:, db, th * 512:(th + 1) * 512], in_=ps,
                    mul=SCALE)

        # release projection-phase SBUF (w/x/stages) before attention allocs
        proj_ctx.close()
        att = ctx.enter_context(tc.tile_pool(name="att", bufs=2))
        attT = ctx.enter_context(tc.tile_pool(name="attT", bufs=1))
        ybuf = ctx.enter_context(tc.tile_pool(name="ybuf", bufs=8))

        # ---------------- Phase ATT ----------------
        # Split into a QK/softmax/transpose pass (consumes kT, which lands
        # early) and an AV pass (consumes v, which lands last) so the tail of
        # the kv exchange hides behind ~30us of score computation.
        probsTs = []
        for k in range(NQB):
            L = 2 * k + 2
            cols = L * P
            nch = (cols + 511) // 512
            widths = [min(512, cols - c * 512) for c in range(nch)]
            probs = att.tile([P, NQB * 2 * P], BF16, tag="probs")
            mx = stat.tile([P, 8], F32, tag="mx")
            negmax = stat.tile([P, 1], F32, tag="negmax")
            sums = stat.tile([P, 8], F32, tag="sums")
            rsum = stat.tile([P, 1], F32, tag="rsum")
            lo = cols - 256
            ch0, off = divmod(lo, 512)
            pss = []
            for ch in range(nch):
                wd = widths[ch]
                ps = psA.tile([P, 512], F32, tag="ps")
                pss.append(ps)
                has_mask = ch == ch0
                for db in range(NDB):
                    nc.tensor.matmul(
                        ps[:, 0:wd], qT_sb[:, db, k * P:(k + 1) * P],
                        kT_sb[:, ch, db, 0:wd],
                        start=(db == 0),
                        stop=(not has_mask and db == NDB - 1))
                if has_mask:
                    # mask folded into the accumulation group: += ident.T @ mask
                    nc.tensor.matmul(
                        ps[:, off:off + 256], ident, mask_sb,
                        start=False, stop=True)
            for ch in range(nch):
                nc.vector.reduce_max(
                    out=mx[:, ch:ch + 1], in_=pss[ch][:, 0:widths[ch]],
                    axis=mybir.AxisListType.X)
            nc.vector.reduce_max(
                out=negmax, in_=mx[:, 0:nch], axis=mybir.AxisListType.X,
                negate=True)
            for ch in range(nch):
                nc.scalar.activation(
                    out=probs[:, ch * 512:ch * 512 + widths[ch]],
                    in_=pss[ch][:, 0:widths[ch]],
                    func=mybir.ActivationFunctionType.Exp,
                    bias=negmax, scale=1.0,
                    accum_out=sums[:, ch:ch + 1])
            probsT = attT.tile([P, L, P], BF16, tag=f"probsT{k}")
            probsTs.append(probsT)
            for j in range(L):
                pt = psT.tile([P, P], BF16)
                nc.tensor.transpose(pt, probs[:, j * P:(j + 1) * P], ident)
                nc.vector.tensor_copy(out=probsT[:, j, :], in_=pt)
            nc.vector.reduce_sum(
                out=rsum, in_=sums[:, 0:nch], axis=mybir.AxisListType.X)
            nc.vector.reciprocal(out=recip_all[:, k:k + 1], in_=rsum)

        for k in range(NQB):
            L = 2 * k + 2
            probsT = probsTs[k]
            y_sb = ybuf.tile([P, C], BF16, tag="y")
            for dh in range(2):
                py = psA.tile([P, 512], F32, tag="ps")
                for j in range(L):
                    nc.tensor.matmul(
                        py, probsT[:, j, :],
                        v_sb[:, j, dh * 512:(dh + 1) * 512],
                        start=(j == 0), stop=(j == L - 1))
                nc.scalar.activation(
                    out=y_sb[:, dh * 512:(dh + 1) * 512], in_=py,
                    func=mybir.ActivationFunctionType.Copy, bias=0.0,
                    scale=recip_all[:, k:k + 1])
            # y out on the Activation hwdge queue: follows the producing
            # activation in the same engine stream, avoiding gpsimd
            # head-of-line blocking behind collective readback waits
            nc.scalar.dma_start(out=out[k * P:(k + 1) * P, :], in_=y_sb)

    return nc


def _host_inputs(x, W):
    """Build per-core input maps (partition-contiguous prearranged)."""
    tril = np.where(
        np.arange(P)[None, :] <= np.arange(P)[:, None], 0.0, NEG
    ).astype(np.float32)
    mask_even = np.concatenate([tril, np.full((P, P), NEG, np.float32)], 1)
    mask_odd = np.concatenate([np.zeros((P, P), np.float32), tril], 1)
    Wb = W.astype(ml_dtypes.bfloat16)
    # wq/wk: [p, db, cb, 128];  wv: [p, dh, cb, 512]
    wq_r = np.ascontiguousarray(
        Wb[:, 0:C].reshape(NCB, P, NDB, P).transpose(1, 2, 0, 3))
    wk_r = np.ascontiguousarray(
        Wb[:, C:2 * C].reshape(NCB, P, NDB, P).transpose(1, 2, 0, 3))
    wv_r = np.ascontiguousarray(
        Wb[:, 2 * C:3 * C].reshape(NCB, P, 2, 512).transpose(1, 2, 0, 3))
    in_maps = []
    for c in range(8):
        b, h = divmod(c, 2)
        xb = x[b].astype(ml_dtypes.bfloat16)        # [T, C]
        qrows = np.concatenate(
            [np.arange((2 * k + h) * P, (2 * k + h + 1) * P) for k in range(NQB)])
        # xq: [p, th, cb, 512]
        xq_r = np.ascontiguousarray(
            xb[qrows].T.reshape(NCB, P, 2, 512).transpose(1, 2, 0, 3))
        # xth (own s-half, transposed): [p, sq, cb, 512]
        xth_r = np.ascontiguousarray(
            xb[h * 1024:(h + 1) * 1024].T.reshape(NCB, P, 2, 512).transpose(
                1, 2, 0, 3))
        in_maps.append({
            "xq": xq_r, "xth": xth_r,
            "wq": wq_r, "wk": wk_r, "wv": wv_r,
            "mask": (mask_even if h == 0 else mask_odd).astype(
                ml_dtypes.bfloat16),
        })
    return in_maps


def _gather(results):
    y = np.zeros((B, T, C), np.float32)
    for c in range(8):
        b, h = divmod(c, 2)
        yc = results[c]["out"]
        for k in range(NQB):
            g = 2 * k + h
            y[b, g * P:(g + 1) * P, :] = yc[k * P:(k + 1) * P, :]
    return y


_SKIP_TYPES = ("InstCall", "InstUnconditionalBranch")


def _wait_limit(inst):
    t = type(inst).__name__
    if t in _SKIP_TYPES:
        return None
    return 1


def _split_excess_waits(nc):
    """HW instruction structs carry few sync-wait slots (1 for compute,
    2 for pseudo-DMA). Move excess waits onto same-engine EventSemaphore
    instructions inserted just before the offender (engines execute their
    stream in order, so this preserves semantics)."""
    fix = 0
    for blk in nc.m.functions[0].blocks:
        out = []
        for inst in blk.instructions:
            lim = _wait_limit(inst)
            si = inst.sync_info
            waits = list(si.on_wait) if si and si.on_wait else []
            if lim is not None and len(waits) > lim:
                for w in waits[:-lim]:
                    fix += 1
                    e = mybir.InstEventSemaphore(
                        name=f"I-waitfix-{fix}", ins=[], outs=[],
                        sync_info=mybir.SyncInfo(on_wait=[w], on_update=[]))
                    e.engine = inst.engine
                    out.append(e)
                si.on_wait = waits[-lim:]
            out.append(inst)
        blk.instructions[:] = out
    return fix


def _audit_waits(nc):
    bad = []
    for blk in nc.m.functions[0].blocks:
        for inst in blk.instructions:
            lim = _wait_limit(inst)
            si = inst.sync_info
            nw = len(si.on_wait) if si and si.on_wait else 0
            if lim is not None and nw > lim:
                bad.append((type(inst).__name__, inst.name, nw))
    return bad


def build_nc_checked(max_tries=6):
    last = None
    for i in range(max_tries):
        nc = build_nc(jitter=i)
        _split_excess_waits(nc)
        bad = _audit_waits(nc)
        if not bad:
            return nc
        last = bad
    raise RuntimeError(f"could not find wait-feasible schedule: {last[:5]}")


_CACHED = {}


def kernel(x, W_kqv):
    x = np.asarray(x, np.float32)
    W = np.asarray(W_kqv, np.float32)
    if "nc" not in _CACHED:
        _CACHED["nc"] = build_nc_checked()
    nc = _CACHED["nc"]
    in_maps = _host_inputs(x, W)
    res = run_bass_kernel_spmd(nc, in_maps, core_ids=list(range(8)))
    return _gather(res.results)


if __name__ == "__main__":
    x = np.random.randn(B, T, C).astype(np.float32)
    W = (np.random.randn(C, 3 * C) * 0.02).astype(np.float32)
    y = kernel(x, W)
    print("kernel ran:", y.shape, y.dtype)
